# revision 5
# baseline (speedup 1.0000x reference)
"""MetaConv1d Trainium2 kernel — v17 (pair-packed matmul1).

Math (per sample n):
  W1c[d, c] = sum_m meta_aug[m, n] * w1aug[m, (d, c)]   (step1, tile-batched)
  G[e, t]   = sum_c W1c[e, c] * x[c, t]                 (matmul1)
  out[t, o] = sum_{e, j} G[e, t+j] * w2[e, (j, o)]      (matmul2, 3 taps)
Host adds the two cheap bias terms (bl linear; w2-bias x channel-sum conv).

v17 changes vs v15:
  - matmul1 is PAIR-PACKED: one K=128 matmul per sample PAIR using a
    block-diagonal lhsT W1cTp[(par*64 + c), (e2*NP + q)] where e2 =
    d + 32*par; even sample's W1cT occupies rows 0:64 / e2 0:32, odd
    rows 64:128 / e2 32:64, zeros elsewhere (zero-DMA'd once at start).
    This halves matmul1 PE time (64 -> 32 cycles/sample... N=128/pair).
  - psG packs 16 samples per PSUM bank [128, 512] (pairs at row-halves
    x 4 column blocks); one evac copy per bank into G_sb.
  - metaT is host-reordered parity-major per tile so the transpose
    evacuation copies have contiguous innermost dims (DVE 2x mode).
  - matmul2 is QUAD-BATCHED: each G_sb column block holds 4 consecutive
    samples' G rows (32 each); one K=128 matmul per tap with a [128, 256]
    block-diagonal w2 rhs computes all 4 samples (N=256 -> 64/sample/tap).
  - out DMA on the SP queue, x DMA on gpsimd: DMA queues overlap
    (concurrent DMAs from different issuing engines run in parallel).
  - hypernet stage is software-pipelined one tile ahead of m1/m2; PE
    warmup matmuls cover the p-state ramp and initial DMA latency.

Sharding: batch*node dim (6624) split evenly over 8 cores (828 each).
"""

import numpy as np
import ml_dtypes

import concourse.mybir as mybir
import concourse.bacc as bacc
from concourse.tile import TileContext
from concourse.bass_utils import run_bass_kernel_spmd

BF16 = mybir.dt.bfloat16
F32 = mybir.dt.float32

B = 32
N = 207
BN = B * N            # 6624
L = 128
C = 64                # in channels
O = 64                # out channels
KK = 3
META = 32
MA = META + 1         # aug (ones row feeds w1 bias in step1)
LOUT = L - KK + 1     # 126
NCORES = 8
PER = BN // NCORES    # 828
NTS = 120             # samples per tile
NP = NTS // 2         # pairs per full tile (W1cTp column stride)


def tiles_for(per, nts=NTS):
    # small first tile (fast pipeline start) and small last tile (short
    # drain); middles of size nts
    if per <= nts:
        return [(0, per)]
    sizes = [48]
    rem = per - 48
    while rem > nts:
        sizes.append(nts)
        rem -= nts
    sizes.append(rem)
    out = []
    n0 = 0
    for s in sizes:
        out.append((n0, s))
        n0 += s
    return out


def build_program(per=PER, nts=NTS):
    assert per % 4 == 0
    npairs = nts // 2
    nc = bacc.Bacc("TRN2", target_bir_lowering=False)

    # x image: partition p = (n%2)*64 + c ; col = (n//2)*L + t
    x_d = nc.dram_tensor("x", (128, (per // 2) * L), BF16, kind="ExternalInput")
    # metaT: per-tile parity-major columns (see host prep)
    metaT_d = nc.dram_tensor("metaT", (MA, per), BF16, kind="ExternalInput")
    # w1augP: [m, d*64 + c] = W1[(c,d), m] (+ w1_b row)
    w1augP_d = nc.dram_tensor("w1augP", (MA, META * C), BF16, kind="ExternalInput")
    # w2 quad-rhs: per tap j a [128, 256] block-diagonal (rows 32s:32s+32
    # -> cols 64s:64s+64 = w2P[:, j-block]); one K=128 matmul computes 4
    # consecutive samples (G_sb stacks their G rows in one column block)
    w2quad_d = nc.dram_tensor("w2quad", (128, KK * 4 * O), BF16, kind="ExternalInput")
    identB_d = nc.dram_tensor("identB", (128, 128), BF16, kind="ExternalInput")
    zeros_d = nc.dram_tensor("zeros", (64, META * npairs), BF16, kind="ExternalInput")
    # out image: [t, n*64 + o] (bf16; host upcasts + adds biases)
    out_d = nc.dram_tensor("out", (LOUT, per * O), BF16, kind="ExternalOutput")

    n_tiles = tiles_for(per, nts)

    with TileContext(nc) as tc:
        with (
            tc.tile_pool(name="const", bufs=1) as cpool,
            tc.tile_pool(name="wpool", bufs=2) as wpool,
            tc.tile_pool(name="xpool", bufs=2) as xpool,
            tc.tile_pool(name="gpool", bufs=2) as gpool,
            tc.tile_pool(name="opool", bufs=2) as opool,
            tc.tile_pool(name="pst", bufs=2, space="PSUM") as pst,
            tc.tile_pool(name="psg", bufs=3, space="PSUM") as psg,
            tc.tile_pool(name="pso", bufs=3, space="PSUM") as pso,
        ):
            # Act issues no DMAs: a DMA occupies the issuing engine for the
            # whole transfer, and Act/DVE are the evacuation workhorses.
            w1augP = cpool.tile([MA, META * C], BF16)
            nc.sync.dma_start(w1augP[:, :], w1augP_d[:, :])
            identB = cpool.tile([128, 128], BF16)
            nc.gpsimd.dma_start(identB[:, :], identB_d[:, :])
            w2quad = cpool.tile([128, KK * 4 * O], BF16)
            w1ctps = []
            for i in range(2):
                w1ctp_i = cpool.tile([128, 64 * npairs], BF16, tag=f"w1ctp{i}")
                w1ctps.append(w1ctp_i)

            def emit_warmup():
                # keep the PE busy from t=0 so the p-state ramp (full clock
                # only after 3us of continuous execution) completes before
                # real matmuls start; also bridges the initial DMA latency.
                wsrc = cpool.tile([128, 512], BF16, tag="warm")
                nc.vector.memset(wsrc[:, :], 0)
                for i in range(6):
                    psW = pst.tile([128, 512], F32, tag="pstile", padded_shape=[128, 512])
                    nc.tensor.matmul(
                        psW[:, :], wsrc[:, 0:128], wsrc[:, :], start=True, stop=True
                    )

            def emit_late_consts():
                # issued after tile 0's input DMAs so they don't delay step1;
                # zeros ride the Pool queue behind the x loads
                nc.sync.dma_start(w2quad[:, :], w2quad_d[:, :])
                for i, t_ in enumerate(w1ctps):
                    eng = nc.gpsimd if i else nc.sync
                    eng.dma_start(t_[64:128, 0 : 32 * npairs], zeros_d[:, :])
                    eng.dma_start(t_[0:64, 32 * npairs : 64 * npairs], zeros_d[:, :])

            def emit_hyper(ti):
                n0, nt = n_tiles[ti]
                np_ = nt // 2
                metaT_sb = wpool.tile([MA, nt], BF16, tag="metaT", padded_shape=[MA, nts])
                nc.sync.dma_start(metaT_sb[:, :], metaT_d[:, n0 : n0 + nt])

                x_sb = xpool.tile(
                    [128, np_ * L], BF16, tag="xsb", padded_shape=[128, npairs * L]
                )
                nc.gpsimd.dma_start(
                    x_sb[:, :], x_d[:, (n0 // 2) * L : ((n0 + nt) // 2) * L]
                )

                W1cTp = w1ctps[ti % 2]

                # step1: W1out[n', (d, c)] batched over the tile
                W1out = wpool.tile(
                    [nt, META * C], BF16, tag="w1out", padded_shape=[nts, META * C]
                )
                for k in range(META * C // 512):
                    ps1 = pst.tile([nt, 512], F32, tag="pstile", padded_shape=[nts, 512])
                    nc.tensor.matmul(
                        ps1[:, :],
                        metaT_sb[:, :],
                        w1augP[:, k * 512 : (k + 1) * 512],
                        start=True,
                        stop=True,
                    )
                    if k % 2:
                        nc.vector.tensor_copy(W1out[:, k * 512 : (k + 1) * 512], ps1[:, :])
                    else:
                        nc.scalar.copy(W1out[:, k * 512 : (k + 1) * 512], ps1[:, :])

                # transpose d-PAIR blocks (nt, 128) -> (128, nt); evac into the
                # block-diagonal e2-major W1cTp layout.
                dstv = W1cTp[:, :].rearrange("p (e q) -> p e q", q=npairs)
                for d0 in range(0, META, 16):
                    psT = pst.tile(
                        [128, 8 * nt], BF16, tag="pstile", padded_shape=[128, 8 * nts]
                    )
                    for k in range(8):
                        nc.tensor.transpose(
                            psT[:, k * nt : (k + 1) * nt],
                            W1out[:, (d0 + 2 * k) * C : (d0 + 2 * k + 2) * C],
                            identB[0:nt, 0:nt],
                        )
                    srcv = psT[:, :].rearrange("p (k n) -> p k n", k=8)
                    for hp in (0, 1):
                        for par in (0, 1):
                            src = srcv[64 * hp : 64 * hp + 64, :, par * np_ : (par + 1) * np_]
                            e0 = d0 + hp + 32 * par
                            dst = dstv[64 * par : 64 * par + 64, e0 : e0 + 15 : 2, 0:np_]
                            nc.vector.tensor_copy(dst, src)

                return (n0, nt, x_sb, W1cTp)

            def emit_m1m2(state, cstep=16):
                n0, nt, x_sb, W1cTp = state
                np_ = nt // 2
                # strided lhsT view: [p, q, e2]
                W1r = W1cTp[:, :].rearrange("p (e q) -> p q e", q=npairs)

                G_sb = gpool.tile(
                    [128, ((np_ + 7) // 8) * 512], BF16, tag="gsb",
                    padded_shape=[128, ((npairs + 7) // 8) * 512],
                )
                out_sb = opool.tile(
                    [LOUT, nt * O], BF16, tag="osb", padded_shape=[LOUT, nts * O]
                )
                out_p = out_sb[:, :].rearrange("t (n z) -> t n z", z=2 * O)

                # c-blocks of pairs (even starts); m1 pipelined one block
                # ahead of m2
                cblocks = [(c0, min(c0 + cstep, np_)) for c0 in range(0, np_, cstep)]

                def emit_m1(c0, ce):
                    for g0 in range(c0, ce, 8):
                        ge = min(g0 + 8, ce)
                        gw = ((ge - g0 + 1) // 2) * 128
                        psG = psg.tile(
                            [128, gw], F32, tag="psG",
                            padded_shape=[128, 512],
                        )
                        for q in range(g0, ge):
                            qq = q - g0
                            nc.tensor.matmul(
                                psG[64 * (qq % 2) : 64 * (qq % 2) + 64,
                                    128 * (qq // 2) : 128 * (qq // 2) + 128],
                                W1r[:, q, :],
                                x_sb[:, q * L : (q + 1) * L],
                                start=True,
                                stop=True,
                            )
                        gb = (g0 // 8) * 512 + ((g0 % 8) // 2) * 128
                        dst = G_sb[:, gb : gb + gw]
                        nc.scalar.copy(dst, psG[:, :])

                def emit_m2(c0, ce):
                    # per QUAD (pairs q0, q0+1 share a G_sb column block with
                    # 4 consecutive samples' G at rows 32s): 3 accumulating
                    # K=128 matmuls with block-diagonal w2 rhs [128, 256]
                    quads = list(range(c0, ce, 2))
                    for b0 in range(0, len(quads), 2):
                        qb = quads[b0 : b0 + 2]
                        psO = pso.tile(
                            [LOUT, len(qb) * 4 * O], F32, tag="psO",
                            padded_shape=[LOUT, 8 * O],
                        )
                        for k, q0 in enumerate(qb):
                            gcol = (q0 // 8) * 512 + ((q0 % 8) // 2) * 128
                            for j in range(KK):
                                nc.tensor.matmul(
                                    psO[:, k * 4 * O : (k + 1) * 4 * O],
                                    G_sb[:, gcol + j : gcol + j + LOUT],
                                    w2quad[:, j * 4 * O : (j + 1) * 4 * O],
                                    start=(j == 0),
                                    stop=(j == KK - 1),
                                )
                        # samples 2*qb[0] .. 2*qb[-1]+3 are consecutive
                        nb = 2 * qb[0]
                        ns_ = 4 * len(qb)
                        dst = out_sb[:, nb * O : (nb + ns_) * O]
                        if (b0 // 2) % 2 == 0:
                            nc.vector.tensor_copy(dst, psO[:, :])
                        else:
                            nc.scalar.copy(dst, psO[:, :])


                emit_m1(*cblocks[0])
                for ci, (c0, ce) in enumerate(cblocks):
                    if ci + 1 < len(cblocks):
                        emit_m1(*cblocks[ci + 1])
                    emit_m2(c0, ce)
                    nc.sync.dma_start(
                        out_d[:, (n0 + 2 * c0) * O : (n0 + 2 * ce) * O],
                        out_sb[:, 2 * c0 * O : 2 * ce * O],
                    )

            # software pipeline: hypernet one tile ahead of m1/m2
            emit_warmup()
            prev = emit_hyper(0)
            emit_late_consts()
            for ti in range(1, len(n_tiles)):
                nxt = emit_hyper(ti)
                emit_m1m2(prev)
                prev = nxt
            emit_m1m2(prev, cstep=8)
    if not nc.is_finalized():
        nc.finalize()
    return nc


def _host_prep(w1_w, w1_b, w2_w, npairs):
    bf = ml_dtypes.bfloat16
    # w1augP[m, (d, c)] = W1[(c*META+d), m]; row 32 = w1_b
    w1 = w1_w.reshape(C, META, META).transpose(2, 1, 0)      # (m, d, c)
    w1b = w1_b.reshape(C, META).T                            # (d, c)
    w1aug = np.concatenate([w1, w1b[None]], axis=0)          # (33, d, c)
    w1augP = w1aug.reshape(MA, META * C)
    # w2P[e, (j, o)] = w2_w[(o*KK+j), e], e < 32; replicated at 4 bases
    w2 = w2_w.reshape(O, KK, META).transpose(2, 1, 0)        # (e, j, o)
    w2P = w2.reshape(META, KK * O)
    w2quad = np.zeros((128, KK * 4 * O), np.float32)
    for j in range(KK):
        for s_ in range(4):
            w2quad[32 * s_ : 32 * s_ + 32, j * 256 + 64 * s_ : j * 256 + 64 * s_ + 64] = \
                w2P[:, j * 64 : (j + 1) * 64]
    identB = np.eye(128, dtype=bf)
    zeros = np.zeros((64, META * npairs), dtype=bf)
    return w1augP.astype(bf), w2quad.astype(bf), identB, zeros


def make_core_inputs(meta, x, w1_w, w1_b, w2_w, w2_b, nts=NTS):
    """meta (per, 32) f32, x (per, L, C) f32 -> input map for one core."""
    bf = ml_dtypes.bfloat16
    per = meta.shape[0]
    w1augP, w2quad, identB, zeros = _host_prep(w1_w, w1_b, w2_w, nts // 2)
    # per-tile parity-major column order for metaT
    perm = []
    for n0, nt in tiles_for(per, nts):
        perm += list(range(n0, n0 + nt, 2)) + list(range(n0 + 1, n0 + nt, 2))
    metaP = meta[np.array(perm)]
    metaT = np.concatenate(
        [metaP.T, np.ones((1, per), np.float32)], axis=0
    ).astype(bf)
    # x image: [ (n%2)*64 + c, (n//2)*L + t ]
    xt = np.ascontiguousarray(x.transpose(0, 2, 1)).astype(bf)   # (per, C, L)
    ximg = xt.reshape(per // 2, 2, C, L).transpose(1, 2, 0, 3).reshape(128, (per // 2) * L)
    return {
        "x": np.ascontiguousarray(ximg),
        "metaT": np.ascontiguousarray(metaT),
        "w1augP": w1augP,
        "w2quad": w2quad,
        "identB": identB,
        "zeros": zeros,
    }


def postprocess_core_output(out_raw, meta, x, w2_b, bl_w=None, bl_b=None):
    """out_raw (LOUT, per*O) bf16 -> (per, LOUT, O) f32 with host bias terms."""
    per = meta.shape[0]
    out = np.asarray(out_raw, dtype=np.float32).reshape(LOUT, per, O).transpose(1, 0, 2)
    # w2 bias term: out[t, o] += sum_j b2[(o,j)] * s[t+j], s = channel sum
    s = x.sum(axis=2)                                        # (per, L)
    b2 = w2_b.reshape(O, KK)                                 # (o, j)
    sw = np.lib.stride_tricks.sliding_window_view(s, KK, axis=1)  # (per, LOUT, KK)
    out = out + sw @ b2.T                                    # (per, LOUT, O)
    if bl_w is not None:
        b = meta @ bl_w.T + bl_b                             # (per, O)
        out = out + b[:, None, :]
    return np.ascontiguousarray(out)


LAST_EXEC_NS = None
_NC_CACHE = {}


def kernel(meta_knowledge, input, w1_w, w1_b, w2_w, w2_b, bl_w, bl_b):
    global LAST_EXEC_NS
    import os

    x_all = np.ascontiguousarray(input.reshape(BN, L, C), dtype=np.float32)

    if PER not in _NC_CACHE:
        _NC_CACHE[PER] = build_program(PER)
    nc = _NC_CACHE[PER]
    in_maps = []
    for i in range(NCORES):
        s = slice(i * PER, (i + 1) * PER)
        in_maps.append(
            make_core_inputs(meta_knowledge[s], x_all[s], w1_w, w1_b, w2_w, w2_b)
        )
    trace = os.environ.get("KM_TRACE", "0") == "1"
    res = run_bass_kernel_spmd(
        nc, in_maps, core_ids=list(range(NCORES)), trace=trace
    )
    if res.exec_time_ns is not None:
        LAST_EXEC_NS = res.exec_time_ns
    outs = []
    for i, r in enumerate(res.results):
        s = slice(i * PER, (i + 1) * PER)
        outs.append(
            postprocess_core_output(
                r["out"], meta_knowledge[s], x_all[s], w2_b, bl_w, bl_b
            )
        )
    out = np.concatenate(outs, axis=0)
    return out.reshape(B, N, LOUT, O)


# revision 6
# speedup vs baseline: 1.0395x; 1.0395x over previous
"""MetaConv1d Trainium2 kernel — v17 (pair-packed matmul1).

Math (per sample n):
  W1c[d, c] = sum_m meta_aug[m, n] * w1aug[m, (d, c)]   (step1, tile-batched)
  G[e, t]   = sum_c W1c[e, c] * x[c, t]                 (matmul1)
  out[t, o] = sum_{e, j} G[e, t+j] * w2[e, (j, o)]      (matmul2, 3 taps)
Host adds the two cheap bias terms (bl linear; w2-bias x channel-sum conv).

v17 changes vs v15:
  - matmul1 is PAIR-PACKED: one K=128 matmul per sample PAIR using a
    block-diagonal lhsT W1cTp[(par*64 + c), (e2*NP + q)] where e2 =
    d + 32*par; even sample's W1cT occupies rows 0:64 / e2 0:32, odd
    rows 64:128 / e2 32:64, zeros elsewhere (zero-DMA'd once at start).
    This halves matmul1 PE time (64 -> 32 cycles/sample... N=128/pair).
  - psG packs 16 samples per PSUM bank [128, 512] (pairs at row-halves
    x 4 column blocks); one evac copy per bank into G_sb.
  - metaT is host-reordered parity-major per tile so the transpose
    evacuation copies have contiguous innermost dims (DVE 2x mode).
  - matmul2 is QUAD-BATCHED: each G_sb column block holds 4 consecutive
    samples' G rows (32 each); one K=128 matmul per tap with a [128, 256]
    block-diagonal w2 rhs computes all 4 samples (N=256 -> 64/sample/tap).
  - out DMA on the SP queue, x DMA on gpsimd: DMA queues overlap
    (concurrent DMAs from different issuing engines run in parallel).
  - hypernet stage is software-pipelined one tile ahead of m1/m2; PE
    warmup matmuls cover the p-state ramp and initial DMA latency.

Sharding: batch*node dim (6624) split evenly over 8 cores (828 each).
"""

import numpy as np
import ml_dtypes

import concourse.mybir as mybir
import concourse.bacc as bacc
from concourse.tile import TileContext
from concourse.bass_utils import run_bass_kernel_spmd

BF16 = mybir.dt.bfloat16
F32 = mybir.dt.float32

B = 32
N = 207
BN = B * N            # 6624
L = 128
C = 64                # in channels
O = 64                # out channels
KK = 3
META = 32
MA = META + 1         # aug (ones row feeds w1 bias in step1)
LOUT = L - KK + 1     # 126
NCORES = 8
PER = BN // NCORES    # 828
NTS = 120             # samples per tile
NP = NTS // 2         # pairs per full tile (W1cTp column stride)


def tiles_for(per, nts=NTS):
    # small first tile (fast pipeline start) and small last tile (short
    # drain); middles of size nts
    if per <= nts:
        return [(0, per)]
    sizes = [48]
    rem = per - 48
    while rem > nts:
        sizes.append(nts)
        rem -= nts
    sizes.append(rem)
    out = []
    n0 = 0
    for s in sizes:
        out.append((n0, s))
        n0 += s
    return out


def build_program(per=PER, nts=NTS):
    assert per % 4 == 0
    npairs = nts // 2
    nc = bacc.Bacc("TRN2", target_bir_lowering=False)

    # x image: partition p = (n%2)*64 + c ; col = (n//2)*L + t
    x_d = nc.dram_tensor("x", (128, (per // 2) * L), BF16, kind="ExternalInput")
    # metaT: per-tile parity-major 128-column blocks, zero-padded so step1
    # fills all 128 W1out rows (pad samples compute garbage weights that
    # land in unread W1cTp columns q >= np_)
    n_tiles0 = tiles_for(per, nts)
    metaT_d = nc.dram_tensor(
        "metaT", (MA, 128 * len(n_tiles0)), BF16, kind="ExternalInput"
    )
    # w1augP: [m, d*64 + c] = W1[(c,d), m] (+ w1_b row)
    w1augP_d = nc.dram_tensor("w1augP", (MA, META * C), BF16, kind="ExternalInput")
    # w2 quad-rhs: per tap j a [128, 256] block-diagonal (rows 32s:32s+32
    # -> cols 64s:64s+64 = w2P[:, j-block]); one K=128 matmul computes 4
    # consecutive samples (G_sb stacks their G rows in one column block)
    w2quad_d = nc.dram_tensor("w2quad", (128, KK * 4 * O), BF16, kind="ExternalInput")
    identB_d = nc.dram_tensor("identB", (128, 128), BF16, kind="ExternalInput")
    zeros_d = nc.dram_tensor("zeros", (64, META * npairs), BF16, kind="ExternalInput")
    # out image: [t, n*64 + o] (bf16; host upcasts + adds biases)
    out_d = nc.dram_tensor("out", (LOUT, per * O), BF16, kind="ExternalOutput")

    n_tiles = tiles_for(per, nts)

    with TileContext(nc) as tc:
        with (
            tc.tile_pool(name="const", bufs=1) as cpool,
            tc.tile_pool(name="wpool", bufs=2) as wpool,
            tc.tile_pool(name="xpool", bufs=2) as xpool,
            tc.tile_pool(name="gpool", bufs=2) as gpool,
            tc.tile_pool(name="opool", bufs=2) as opool,
            tc.tile_pool(name="pst", bufs=2, space="PSUM") as pst,
            tc.tile_pool(name="psg", bufs=3, space="PSUM") as psg,
            tc.tile_pool(name="pso", bufs=3, space="PSUM") as pso,
        ):
            # Act issues no DMAs: a DMA occupies the issuing engine for the
            # whole transfer, and Act/DVE are the evacuation workhorses.
            w1augP = cpool.tile([MA, META * C], BF16)
            nc.sync.dma_start(w1augP[:, :], w1augP_d[:, :])
            identB = cpool.tile([128, 128], BF16)
            nc.gpsimd.dma_start(identB[:, :], identB_d[:, :])
            w2quad = cpool.tile([128, KK * 4 * O], BF16)
            w1ctps = []
            for i in range(2):
                w1ctp_i = cpool.tile([128, 64 * npairs], BF16, tag=f"w1ctp{i}")
                w1ctps.append(w1ctp_i)

            def emit_warmup():
                # keep the PE busy from t=0 so the p-state ramp (full clock
                # only after 3us of continuous execution) completes before
                # real matmuls start; also bridges the initial DMA latency.
                wsrc = cpool.tile([128, 512], BF16, tag="warm")
                nc.vector.memset(wsrc[:, :], 0)
                for i in range(6):
                    psW = pst.tile([128, 512], F32, tag="pstile", padded_shape=[128, 512])
                    nc.tensor.matmul(
                        psW[:, :], wsrc[:, 0:128], wsrc[:, :], start=True, stop=True
                    )

            def emit_late_consts():
                # issued after tile 0's input DMAs so they don't delay step1;
                # zeros ride the Pool queue behind the x loads
                nc.sync.dma_start(w2quad[:, :], w2quad_d[:, :])
                for i, t_ in enumerate(w1ctps):
                    eng = nc.gpsimd if i else nc.sync
                    eng.dma_start(t_[64:128, 0 : 32 * npairs], zeros_d[:, :])
                    eng.dma_start(t_[0:64, 32 * npairs : 64 * npairs], zeros_d[:, :])

            def emit_hyper(ti):
                n0, nt = n_tiles[ti]
                np_ = nt // 2
                metaT_sb = wpool.tile([MA, 128], BF16, tag="metaT", padded_shape=[MA, 128])
                nc.sync.dma_start(metaT_sb[:, :], metaT_d[:, 128 * ti : 128 * ti + 128])

                x_sb = xpool.tile(
                    [128, np_ * L], BF16, tag="xsb", padded_shape=[128, npairs * L]
                )
                nc.gpsimd.dma_start(
                    x_sb[:, :], x_d[:, (n0 // 2) * L : ((n0 + nt) // 2) * L]
                )

                W1cTp = w1ctps[ti % 2]

                # step1: W1out[n', (d, c)] batched over the tile.
                # Fixed 128 partitions: the DMA-transpose reads all rows
                # (pad rows are zeroed once at startup).
                W1out = wpool.tile(
                    [128, META * C], BF16, tag="w1out", padded_shape=[128, META * C]
                )
                for k in range(META * C // 512):
                    ps1 = pst.tile([128, 512], F32, tag="pstile", padded_shape=[128, 512])
                    nc.tensor.matmul(
                        ps1[:, :],
                        metaT_sb[:, :],
                        w1augP[:, k * 512 : (k + 1) * 512],
                        start=True,
                        stop=True,
                    )
                    if k % 2:
                        nc.vector.tensor_copy(W1out[:, k * 512 : (k + 1) * 512], ps1[:, :])
                    else:
                        nc.scalar.copy(W1out[:, k * 512 : (k + 1) * 512], ps1[:, :])

                # transpose W1out (n', (d, c)) -> (c + 64*dpar, dpair, n');
                # evac into the block-diagonal e2-major W1cTp layout.
                dstv = W1cTp[:, :].rearrange("p (e q) -> p e q", q=npairs)
                if ti == 0:
                    # PE-transpose path: keeps tile 0's pre-m1 chain short
                    for d0 in range(0, META, 16):
                        psT = pst.tile(
                            [128, 8 * nt], BF16, tag="pstile", padded_shape=[128, 8 * nts]
                        )
                        for k in range(8):
                            nc.tensor.transpose(
                                psT[:, k * nt : (k + 1) * nt],
                                W1out[0:nt, (d0 + 2 * k) * C : (d0 + 2 * k + 2) * C],
                                identB[0:nt, 0:nt],
                            )
                        srcv = psT[:, :].rearrange("p (k n) -> p k n", k=8)
                        for hp in (0, 1):
                            for par in (0, 1):
                                src = srcv[64 * hp : 64 * hp + 64, :, par * np_ : (par + 1) * np_]
                                e0 = d0 + hp + 32 * par
                                dst = dstv[64 * par : 64 * par + 64, e0 : e0 + 15 : 2, 0:np_]
                                nc.vector.tensor_copy(dst, src)
                else:
                    # xbar DMA-transpose: one instruction moves the whole
                    # tile's transpose to the DMA engines (16x128 tiles);
                    # out[p, b, r] = W1out[r, 128b + p]
                    psT_sb = wpool.tile(
                        [128, 16 * 128], BF16, tag="ptsb", padded_shape=[128, 16 * 128]
                    )
                    nc.sync.dma_start_transpose(
                        psT_sb[:, :].rearrange("p (k n) -> p k n", k=16),
                        W1out[:, :],
                    )
                    srcv = psT_sb[:, :].rearrange("p (k n) -> p k n", k=16)
                    for hp in (0, 1):
                        for par in (0, 1):
                            src = srcv[64 * hp : 64 * hp + 64, :, par * np_ : (par + 1) * np_]
                            e0 = hp + 32 * par
                            dst = dstv[64 * par : 64 * par + 64, e0 : e0 + 31 : 2, 0:np_]
                            nc.vector.tensor_copy(dst, src)

                return (n0, nt, x_sb, W1cTp)

            def emit_m1m2(state, cstep=16):
                n0, nt, x_sb, W1cTp = state
                np_ = nt // 2
                # strided lhsT view: [p, q, e2]
                W1r = W1cTp[:, :].rearrange("p (e q) -> p q e", q=npairs)

                G_sb = gpool.tile(
                    [128, ((np_ + 7) // 8) * 512], BF16, tag="gsb",
                    padded_shape=[128, ((npairs + 7) // 8) * 512],
                )
                out_sb = opool.tile(
                    [LOUT, nt * O], BF16, tag="osb", padded_shape=[LOUT, nts * O]
                )
                out_p = out_sb[:, :].rearrange("t (n z) -> t n z", z=2 * O)

                # c-blocks of pairs (even starts); m1 pipelined one block
                # ahead of m2
                cblocks = [(c0, min(c0 + cstep, np_)) for c0 in range(0, np_, cstep)]

                def emit_m1(c0, ce):
                    for g0 in range(c0, ce, 8):
                        ge = min(g0 + 8, ce)
                        gw = ((ge - g0 + 1) // 2) * 128
                        psG = psg.tile(
                            [128, gw], F32, tag="psG",
                            padded_shape=[128, 512],
                        )
                        for q in range(g0, ge):
                            qq = q - g0
                            nc.tensor.matmul(
                                psG[64 * (qq % 2) : 64 * (qq % 2) + 64,
                                    128 * (qq // 2) : 128 * (qq // 2) + 128],
                                W1r[:, q, :],
                                x_sb[:, q * L : (q + 1) * L],
                                start=True,
                                stop=True,
                            )
                        gb = (g0 // 8) * 512 + ((g0 % 8) // 2) * 128
                        dst = G_sb[:, gb : gb + gw]
                        nc.scalar.copy(dst, psG[:, :])

                def emit_m2(c0, ce):
                    # per QUAD (pairs q0, q0+1 share a G_sb column block with
                    # 4 consecutive samples' G at rows 32s): 3 accumulating
                    # K=128 matmuls with block-diagonal w2 rhs [128, 256]
                    quads = list(range(c0, ce, 2))
                    for b0 in range(0, len(quads), 2):
                        qb = quads[b0 : b0 + 2]
                        psO = pso.tile(
                            [LOUT, len(qb) * 4 * O], F32, tag="psO",
                            padded_shape=[LOUT, 8 * O],
                        )
                        for k, q0 in enumerate(qb):
                            gcol = (q0 // 8) * 512 + ((q0 % 8) // 2) * 128
                            for j in range(KK):
                                nc.tensor.matmul(
                                    psO[:, k * 4 * O : (k + 1) * 4 * O],
                                    G_sb[:, gcol + j : gcol + j + LOUT],
                                    w2quad[:, j * 4 * O : (j + 1) * 4 * O],
                                    start=(j == 0),
                                    stop=(j == KK - 1),
                                )
                        # samples 2*qb[0] .. 2*qb[-1]+3 are consecutive
                        nb = 2 * qb[0]
                        ns_ = 4 * len(qb)
                        dst = out_sb[:, nb * O : (nb + ns_) * O]
                        if (b0 // 2) % 2 == 0:
                            nc.vector.tensor_copy(dst, psO[:, :])
                        else:
                            nc.scalar.copy(dst, psO[:, :])


                emit_m1(*cblocks[0])
                for ci, (c0, ce) in enumerate(cblocks):
                    if ci + 1 < len(cblocks):
                        emit_m1(*cblocks[ci + 1])
                    emit_m2(c0, ce)
                    nc.sync.dma_start(
                        out_d[:, (n0 + 2 * c0) * O : (n0 + 2 * ce) * O],
                        out_sb[:, 2 * c0 * O : 2 * ce * O],
                    )

            # software pipeline: hypernet one tile ahead of m1/m2
            emit_warmup()
            prev = emit_hyper(0)
            emit_late_consts()
            for ti in range(1, len(n_tiles)):
                nxt = emit_hyper(ti)
                emit_m1m2(prev)
                prev = nxt
            emit_m1m2(prev, cstep=8)
    if not nc.is_finalized():
        nc.finalize()
    return nc


def _host_prep(w1_w, w1_b, w2_w, npairs):
    bf = ml_dtypes.bfloat16
    # w1augP[m, (d, c)] = W1[(c*META+d), m]; row 32 = w1_b
    w1 = w1_w.reshape(C, META, META).transpose(2, 1, 0)      # (m, d, c)
    w1b = w1_b.reshape(C, META).T                            # (d, c)
    w1aug = np.concatenate([w1, w1b[None]], axis=0)          # (33, d, c)
    w1augP = w1aug.reshape(MA, META * C)
    # w2P[e, (j, o)] = w2_w[(o*KK+j), e], e < 32; replicated at 4 bases
    w2 = w2_w.reshape(O, KK, META).transpose(2, 1, 0)        # (e, j, o)
    w2P = w2.reshape(META, KK * O)
    w2quad = np.zeros((128, KK * 4 * O), np.float32)
    for j in range(KK):
        for s_ in range(4):
            w2quad[32 * s_ : 32 * s_ + 32, j * 256 + 64 * s_ : j * 256 + 64 * s_ + 64] = \
                w2P[:, j * 64 : (j + 1) * 64]
    identB = np.eye(128, dtype=bf)
    zeros = np.zeros((64, META * npairs), dtype=bf)
    return w1augP.astype(bf), w2quad.astype(bf), identB, zeros


def make_core_inputs(meta, x, w1_w, w1_b, w2_w, w2_b, nts=NTS):
    """meta (per, 32) f32, x (per, L, C) f32 -> input map for one core."""
    bf = ml_dtypes.bfloat16
    per = meta.shape[0]
    w1augP, w2quad, identB, zeros = _host_prep(w1_w, w1_b, w2_w, nts // 2)
    # per-tile parity-major column order for metaT
    tls = tiles_for(per, nts)
    metaT = np.zeros((MA, 128 * len(tls)), np.float32)
    for ti, (n0, nt) in enumerate(tls):
        perm = list(range(n0, n0 + nt, 2)) + list(range(n0 + 1, n0 + nt, 2))
        metaT[:META, 128 * ti : 128 * ti + nt] = meta[np.array(perm)].T
        metaT[META, 128 * ti : 128 * ti + nt] = 1.0
    metaT = metaT.astype(bf)
    # x image: [ (n%2)*64 + c, (n//2)*L + t ]
    xt = np.ascontiguousarray(x.transpose(0, 2, 1)).astype(bf)   # (per, C, L)
    ximg = xt.reshape(per // 2, 2, C, L).transpose(1, 2, 0, 3).reshape(128, (per // 2) * L)
    return {
        "x": np.ascontiguousarray(ximg),
        "metaT": np.ascontiguousarray(metaT),
        "w1augP": w1augP,
        "w2quad": w2quad,
        "identB": identB,
        "zeros": zeros,
    }


def postprocess_core_output(out_raw, meta, x, w2_b, bl_w=None, bl_b=None):
    """out_raw (LOUT, per*O) bf16 -> (per, LOUT, O) f32 with host bias terms."""
    per = meta.shape[0]
    out = np.asarray(out_raw, dtype=np.float32).reshape(LOUT, per, O).transpose(1, 0, 2)
    # w2 bias term: out[t, o] += sum_j b2[(o,j)] * s[t+j], s = channel sum
    s = x.sum(axis=2)                                        # (per, L)
    b2 = w2_b.reshape(O, KK)                                 # (o, j)
    sw = np.lib.stride_tricks.sliding_window_view(s, KK, axis=1)  # (per, LOUT, KK)
    out = out + sw @ b2.T                                    # (per, LOUT, O)
    if bl_w is not None:
        b = meta @ bl_w.T + bl_b                             # (per, O)
        out = out + b[:, None, :]
    return np.ascontiguousarray(out)


LAST_EXEC_NS = None
_NC_CACHE = {}


def kernel(meta_knowledge, input, w1_w, w1_b, w2_w, w2_b, bl_w, bl_b):
    global LAST_EXEC_NS
    import os

    x_all = np.ascontiguousarray(input.reshape(BN, L, C), dtype=np.float32)

    if PER not in _NC_CACHE:
        _NC_CACHE[PER] = build_program(PER)
    nc = _NC_CACHE[PER]
    in_maps = []
    for i in range(NCORES):
        s = slice(i * PER, (i + 1) * PER)
        in_maps.append(
            make_core_inputs(meta_knowledge[s], x_all[s], w1_w, w1_b, w2_w, w2_b)
        )
    trace = os.environ.get("KM_TRACE", "0") == "1"
    res = run_bass_kernel_spmd(
        nc, in_maps, core_ids=list(range(NCORES)), trace=trace
    )
    if res.exec_time_ns is not None:
        LAST_EXEC_NS = res.exec_time_ns
    outs = []
    for i, r in enumerate(res.results):
        s = slice(i * PER, (i + 1) * PER)
        outs.append(
            postprocess_core_output(
                r["out"], meta_knowledge[s], x_all[s], w2_b, bl_w, bl_b
            )
        )
    out = np.concatenate(outs, axis=0)
    return out.reshape(B, N, LOUT, O)


# revision 7
# speedup vs baseline: 1.0490x; 1.0091x over previous
"""MetaConv1d Trainium2 kernel — v17 (pair-packed matmul1).

Math (per sample n):
  W1c[d, c] = sum_m meta_aug[m, n] * w1aug[m, (d, c)]   (step1, tile-batched)
  G[e, t]   = sum_c W1c[e, c] * x[c, t]                 (matmul1)
  out[t, o] = sum_{e, j} G[e, t+j] * w2[e, (j, o)]      (matmul2, 3 taps)
Host adds the two cheap bias terms (bl linear; w2-bias x channel-sum conv).

v17 changes vs v15:
  - matmul1 is PAIR-PACKED: one K=128 matmul per sample PAIR using a
    block-diagonal lhsT W1cTp[(par*64 + c), (e2*NP + q)] where e2 =
    d + 32*par; even sample's W1cT occupies rows 0:64 / e2 0:32, odd
    rows 64:128 / e2 32:64, zeros elsewhere (zero-DMA'd once at start).
    This halves matmul1 PE time (64 -> 32 cycles/sample... N=128/pair).
  - psG packs 16 samples per PSUM bank [128, 512] (pairs at row-halves
    x 4 column blocks); one evac copy per bank into G_sb.
  - metaT is host-reordered parity-major per tile so the transpose
    evacuation copies have contiguous innermost dims (DVE 2x mode).
  - matmul2 is QUAD-BATCHED: each G_sb column block holds 4 consecutive
    samples' G rows (32 each); one K=128 matmul per tap with a [128, 256]
    block-diagonal w2 rhs computes all 4 samples (N=256 -> 64/sample/tap).
  - out DMA on the SP queue, x DMA on gpsimd: DMA queues overlap
    (concurrent DMAs from different issuing engines run in parallel).
  - hypernet stage is software-pipelined one tile ahead of m1/m2; PE
    warmup matmuls cover the p-state ramp and initial DMA latency.

Sharding: batch*node dim (6624) split evenly over 8 cores (828 each).
"""

import numpy as np
import ml_dtypes

import concourse.mybir as mybir
import concourse.bacc as bacc
from concourse.tile import TileContext
from concourse.bass_utils import run_bass_kernel_spmd

BF16 = mybir.dt.bfloat16
F32 = mybir.dt.float32

B = 32
N = 207
BN = B * N            # 6624
L = 128
C = 64                # in channels
O = 64                # out channels
KK = 3
META = 32
MA = META + 1         # aug (ones row feeds w1 bias in step1)
LOUT = L - KK + 1     # 126
NCORES = 8
PER = BN // NCORES    # 828
NTS = 128             # samples per tile
NP = NTS // 2         # pairs per full tile (W1cTp column stride)


def tiles_for(per, nts=NTS):
    # small first tile (fast pipeline start) and small last tile (short
    # drain); middles of size nts
    if per <= nts:
        return [(0, per)]
    rem = per
    k = 0
    while rem - nts >= 48:
        rem -= nts
        k += 1
    sizes = [rem] + [nts] * k
    out = []
    n0 = 0
    for s in sizes:
        out.append((n0, s))
        n0 += s
    return out


def build_program(per=PER, nts=NTS):
    assert per % 4 == 0
    npairs = nts // 2
    nc = bacc.Bacc("TRN2", target_bir_lowering=False)

    # x image: partition p = (n%2)*64 + c ; col = (n//2)*L + t
    x_d = nc.dram_tensor("x", (128, (per // 2) * L), BF16, kind="ExternalInput")
    # metaT: per-tile parity-major 128-column blocks, zero-padded so step1
    # fills all 128 W1out rows (pad samples compute garbage weights that
    # land in unread W1cTp columns q >= np_)
    n_tiles0 = tiles_for(per, nts)
    metaT_d = nc.dram_tensor(
        "metaT", (MA, 128 * len(n_tiles0)), BF16, kind="ExternalInput"
    )
    # w1augP: [m, d*64 + c] = W1[(c,d), m] (+ w1_b row)
    w1augP_d = nc.dram_tensor("w1augP", (MA, META * C), BF16, kind="ExternalInput")
    # w2 quad-rhs: per tap j a [128, 256] block-diagonal (rows 32s:32s+32
    # -> cols 64s:64s+64 = w2P[:, j-block]); one K=128 matmul computes 4
    # consecutive samples (G_sb stacks their G rows in one column block)
    w2quad_d = nc.dram_tensor("w2quad", (128, KK * 4 * O), BF16, kind="ExternalInput")
    identB_d = nc.dram_tensor("identB", (128, 128), BF16, kind="ExternalInput")
    zeros_d = nc.dram_tensor("zeros", (64, META * npairs), BF16, kind="ExternalInput")
    # out image: [t, n*64 + o] (bf16; host upcasts + adds biases)
    out_d = nc.dram_tensor("out", (LOUT, per * O), BF16, kind="ExternalOutput")

    n_tiles = tiles_for(per, nts)

    with TileContext(nc) as tc:
        with (
            tc.tile_pool(name="const", bufs=1) as cpool,
            tc.tile_pool(name="wpool", bufs=2) as wpool,
            tc.tile_pool(name="xpool", bufs=2) as xpool,
            tc.tile_pool(name="gpool", bufs=2) as gpool,
            tc.tile_pool(name="opool", bufs=2) as opool,
            tc.tile_pool(name="pst", bufs=2, space="PSUM") as pst,
            tc.tile_pool(name="psg", bufs=3, space="PSUM") as psg,
            tc.tile_pool(name="pso", bufs=3, space="PSUM") as pso,
        ):
            # Act issues no DMAs: a DMA occupies the issuing engine for the
            # whole transfer, and Act/DVE are the evacuation workhorses.
            w1augP = cpool.tile([MA, META * C], BF16)
            nc.sync.dma_start(w1augP[:, :], w1augP_d[:, :])
            identB = cpool.tile([128, 128], BF16)
            nc.gpsimd.dma_start(identB[:, :], identB_d[:, :])
            w2quad = cpool.tile([128, KK * 4 * O], BF16)
            w1ctps = []
            for i in range(2):
                w1ctp_i = cpool.tile([128, 64 * npairs], BF16, tag=f"w1ctp{i}")
                w1ctps.append(w1ctp_i)

            def emit_warmup():
                # keep the PE busy from t=0 so the p-state ramp (full clock
                # only after 3us of continuous execution) completes before
                # real matmuls start; also bridges the initial DMA latency.
                wsrc = cpool.tile([128, 512], BF16, tag="warm")
                nc.vector.memset(wsrc[:, :], 0)
                for i in range(6):
                    psW = pst.tile([128, 512], F32, tag="pstile", padded_shape=[128, 512])
                    nc.tensor.matmul(
                        psW[:, :], wsrc[:, 0:128], wsrc[:, :], start=True, stop=True
                    )

            def emit_late_consts():
                # issued after tile 0's input DMAs so they don't delay step1;
                # zeros ride the Pool queue behind the x loads
                nc.sync.dma_start(w2quad[:, :], w2quad_d[:, :])
                for i, t_ in enumerate(w1ctps):
                    eng = nc.gpsimd if i else nc.sync
                    eng.dma_start(t_[64:128, 0 : 32 * npairs], zeros_d[:, :])
                    eng.dma_start(t_[0:64, 32 * npairs : 64 * npairs], zeros_d[:, :])

            def emit_hyper(ti):
                n0, nt = n_tiles[ti]
                np_ = nt // 2
                metaT_sb = wpool.tile([MA, 128], BF16, tag="metaT", padded_shape=[MA, 128])
                nc.sync.dma_start(metaT_sb[:, :], metaT_d[:, 128 * ti : 128 * ti + 128])

                x_sb = xpool.tile(
                    [128, np_ * L], BF16, tag="xsb", padded_shape=[128, npairs * L]
                )
                nc.gpsimd.dma_start(
                    x_sb[:, :], x_d[:, (n0 // 2) * L : ((n0 + nt) // 2) * L]
                )

                W1cTp = w1ctps[ti % 2]

                # step1: W1out[n', (d, c)] batched over the tile.
                # Fixed 128 partitions: the DMA-transpose reads all rows
                # (pad rows are zeroed once at startup).
                W1out = wpool.tile(
                    [128, META * C], BF16, tag="w1out", padded_shape=[128, META * C]
                )
                for k in range(META * C // 512):
                    ps1 = pst.tile([128, 512], F32, tag="pstile", padded_shape=[128, 512])
                    nc.tensor.matmul(
                        ps1[:, :],
                        metaT_sb[:, :],
                        w1augP[:, k * 512 : (k + 1) * 512],
                        start=True,
                        stop=True,
                    )
                    if k % 2:
                        nc.vector.tensor_copy(W1out[:, k * 512 : (k + 1) * 512], ps1[:, :])
                    else:
                        nc.scalar.copy(W1out[:, k * 512 : (k + 1) * 512], ps1[:, :])

                # transpose W1out (n', (d, c)) -> (c + 64*dpar, dpair, n');
                # evac into the block-diagonal e2-major W1cTp layout.
                dstv = W1cTp[:, :].rearrange("p (e q) -> p e q", q=npairs)
                if ti == 0:
                    # PE-transpose path: keeps tile 0's pre-m1 chain short
                    for d0 in range(0, META, 16):
                        psT = pst.tile(
                            [128, 8 * nt], BF16, tag="pstile", padded_shape=[128, 8 * nts]
                        )
                        for k in range(8):
                            nc.tensor.transpose(
                                psT[:, k * nt : (k + 1) * nt],
                                W1out[0:nt, (d0 + 2 * k) * C : (d0 + 2 * k + 2) * C],
                                identB[0:nt, 0:nt],
                            )
                        srcv = psT[:, :].rearrange("p (k n) -> p k n", k=8)
                        for hp in (0, 1):
                            for par in (0, 1):
                                src = srcv[64 * hp : 64 * hp + 64, :, par * np_ : (par + 1) * np_]
                                e0 = d0 + hp + 32 * par
                                dst = dstv[64 * par : 64 * par + 64, e0 : e0 + 15 : 2, 0:np_]
                                nc.vector.tensor_copy(dst, src)
                else:
                    # xbar DMA-transpose: one instruction moves the whole
                    # tile's transpose to the DMA engines (16x128 tiles);
                    # out[p, b, r] = W1out[r, 128b + p]
                    psT_sb = wpool.tile(
                        [128, 16 * 128], BF16, tag="ptsb", padded_shape=[128, 16 * 128]
                    )
                    nc.sync.dma_start_transpose(
                        psT_sb[:, :].rearrange("p (k n) -> p k n", k=16),
                        W1out[:, :],
                    )
                    srcv = psT_sb[:, :].rearrange("p (k n) -> p k n", k=16)
                    for hp in (0, 1):
                        for par in (0, 1):
                            src = srcv[64 * hp : 64 * hp + 64, :, par * np_ : (par + 1) * np_]
                            e0 = hp + 32 * par
                            dst = dstv[64 * par : 64 * par + 64, e0 : e0 + 31 : 2, 0:np_]
                            nc.vector.tensor_copy(dst, src)

                return (n0, nt, x_sb, W1cTp)

            def emit_m1m2(state, cstep=16):
                n0, nt, x_sb, W1cTp = state
                np_ = nt // 2
                # strided lhsT view: [p, q, e2]
                W1r = W1cTp[:, :].rearrange("p (e q) -> p q e", q=npairs)

                G_sb = gpool.tile(
                    [128, ((np_ + 7) // 8) * 512], BF16, tag="gsb",
                    padded_shape=[128, ((npairs + 7) // 8) * 512],
                )
                out_sb = opool.tile(
                    [LOUT, nt * O], BF16, tag="osb", padded_shape=[LOUT, nts * O]
                )
                out_p = out_sb[:, :].rearrange("t (n z) -> t n z", z=2 * O)

                # c-blocks of pairs (even starts); m1 pipelined one block
                # ahead of m2
                cblocks = [(c0, min(c0 + cstep, np_)) for c0 in range(0, np_, cstep)]

                def emit_m1(c0, ce):
                    for g0 in range(c0, ce, 8):
                        ge = min(g0 + 8, ce)
                        gw = ((ge - g0 + 1) // 2) * 128
                        psG = psg.tile(
                            [128, gw], F32, tag="psG",
                            padded_shape=[128, 512],
                        )
                        for q in range(g0, ge):
                            qq = q - g0
                            nc.tensor.matmul(
                                psG[64 * (qq % 2) : 64 * (qq % 2) + 64,
                                    128 * (qq // 2) : 128 * (qq // 2) + 128],
                                W1r[:, q, :],
                                x_sb[:, q * L : (q + 1) * L],
                                start=True,
                                stop=True,
                            )
                        gb = (g0 // 8) * 512 + ((g0 % 8) // 2) * 128
                        dst = G_sb[:, gb : gb + gw]
                        nc.scalar.copy(dst, psG[:, :])

                def emit_m2(c0, ce):
                    # per QUAD (pairs q0, q0+1 share a G_sb column block with
                    # 4 consecutive samples' G at rows 32s): 3 accumulating
                    # K=128 matmuls with block-diagonal w2 rhs [128, 256]
                    quads = list(range(c0, ce, 2))
                    for b0 in range(0, len(quads), 2):
                        qb = quads[b0 : b0 + 2]
                        psO = pso.tile(
                            [LOUT, len(qb) * 4 * O], F32, tag="psO",
                            padded_shape=[LOUT, 8 * O],
                        )
                        for k, q0 in enumerate(qb):
                            gcol = (q0 // 8) * 512 + ((q0 % 8) // 2) * 128
                            for j in range(KK):
                                nc.tensor.matmul(
                                    psO[:, k * 4 * O : (k + 1) * 4 * O],
                                    G_sb[:, gcol + j : gcol + j + LOUT],
                                    w2quad[:, j * 4 * O : (j + 1) * 4 * O],
                                    start=(j == 0),
                                    stop=(j == KK - 1),
                                )
                        # samples 2*qb[0] .. 2*qb[-1]+3 are consecutive
                        nb = 2 * qb[0]
                        ns_ = 4 * len(qb)
                        dst = out_sb[:, nb * O : (nb + ns_) * O]
                        if (b0 // 2) % 2 == 0:
                            nc.vector.tensor_copy(dst, psO[:, :])
                        else:
                            nc.scalar.copy(dst, psO[:, :])


                emit_m1(*cblocks[0])
                for ci, (c0, ce) in enumerate(cblocks):
                    if ci + 1 < len(cblocks):
                        emit_m1(*cblocks[ci + 1])
                    emit_m2(c0, ce)
                    nc.sync.dma_start(
                        out_d[:, (n0 + 2 * c0) * O : (n0 + 2 * ce) * O],
                        out_sb[:, 2 * c0 * O : 2 * ce * O],
                    )

            # software pipeline: hypernet one tile ahead of m1/m2
            emit_warmup()
            prev = emit_hyper(0)
            emit_late_consts()
            for ti in range(1, len(n_tiles)):
                nxt = emit_hyper(ti)
                emit_m1m2(prev)
                prev = nxt
            emit_m1m2(prev, cstep=8)
    if not nc.is_finalized():
        nc.finalize()
    return nc


def _host_prep(w1_w, w1_b, w2_w, npairs):
    bf = ml_dtypes.bfloat16
    # w1augP[m, (d, c)] = W1[(c*META+d), m]; row 32 = w1_b
    w1 = w1_w.reshape(C, META, META).transpose(2, 1, 0)      # (m, d, c)
    w1b = w1_b.reshape(C, META).T                            # (d, c)
    w1aug = np.concatenate([w1, w1b[None]], axis=0)          # (33, d, c)
    w1augP = w1aug.reshape(MA, META * C)
    # w2P[e, (j, o)] = w2_w[(o*KK+j), e], e < 32; replicated at 4 bases
    w2 = w2_w.reshape(O, KK, META).transpose(2, 1, 0)        # (e, j, o)
    w2P = w2.reshape(META, KK * O)
    w2quad = np.zeros((128, KK * 4 * O), np.float32)
    for j in range(KK):
        for s_ in range(4):
            w2quad[32 * s_ : 32 * s_ + 32, j * 256 + 64 * s_ : j * 256 + 64 * s_ + 64] = \
                w2P[:, j * 64 : (j + 1) * 64]
    identB = np.eye(128, dtype=bf)
    zeros = np.zeros((64, META * npairs), dtype=bf)
    return w1augP.astype(bf), w2quad.astype(bf), identB, zeros


def make_core_inputs(meta, x, w1_w, w1_b, w2_w, w2_b, nts=NTS):
    """meta (per, 32) f32, x (per, L, C) f32 -> input map for one core."""
    bf = ml_dtypes.bfloat16
    per = meta.shape[0]
    w1augP, w2quad, identB, zeros = _host_prep(w1_w, w1_b, w2_w, nts // 2)
    # per-tile parity-major column order for metaT
    tls = tiles_for(per, nts)
    metaT = np.zeros((MA, 128 * len(tls)), np.float32)
    for ti, (n0, nt) in enumerate(tls):
        perm = list(range(n0, n0 + nt, 2)) + list(range(n0 + 1, n0 + nt, 2))
        metaT[:META, 128 * ti : 128 * ti + nt] = meta[np.array(perm)].T
        metaT[META, 128 * ti : 128 * ti + nt] = 1.0
    metaT = metaT.astype(bf)
    # x image: [ (n%2)*64 + c, (n//2)*L + t ]
    xt = np.ascontiguousarray(x.transpose(0, 2, 1)).astype(bf)   # (per, C, L)
    ximg = xt.reshape(per // 2, 2, C, L).transpose(1, 2, 0, 3).reshape(128, (per // 2) * L)
    return {
        "x": np.ascontiguousarray(ximg),
        "metaT": np.ascontiguousarray(metaT),
        "w1augP": w1augP,
        "w2quad": w2quad,
        "identB": identB,
        "zeros": zeros,
    }


def postprocess_core_output(out_raw, meta, x, w2_b, bl_w=None, bl_b=None):
    """out_raw (LOUT, per*O) bf16 -> (per, LOUT, O) f32 with host bias terms."""
    per = meta.shape[0]
    out = np.asarray(out_raw, dtype=np.float32).reshape(LOUT, per, O).transpose(1, 0, 2)
    # w2 bias term: out[t, o] += sum_j b2[(o,j)] * s[t+j], s = channel sum
    s = x.sum(axis=2)                                        # (per, L)
    b2 = w2_b.reshape(O, KK)                                 # (o, j)
    sw = np.lib.stride_tricks.sliding_window_view(s, KK, axis=1)  # (per, LOUT, KK)
    out = out + sw @ b2.T                                    # (per, LOUT, O)
    if bl_w is not None:
        b = meta @ bl_w.T + bl_b                             # (per, O)
        out = out + b[:, None, :]
    return np.ascontiguousarray(out)


LAST_EXEC_NS = None
_NC_CACHE = {}


def kernel(meta_knowledge, input, w1_w, w1_b, w2_w, w2_b, bl_w, bl_b):
    global LAST_EXEC_NS
    import os

    x_all = np.ascontiguousarray(input.reshape(BN, L, C), dtype=np.float32)

    if PER not in _NC_CACHE:
        _NC_CACHE[PER] = build_program(PER)
    nc = _NC_CACHE[PER]
    in_maps = []
    for i in range(NCORES):
        s = slice(i * PER, (i + 1) * PER)
        in_maps.append(
            make_core_inputs(meta_knowledge[s], x_all[s], w1_w, w1_b, w2_w, w2_b)
        )
    trace = os.environ.get("KM_TRACE", "0") == "1"
    res = run_bass_kernel_spmd(
        nc, in_maps, core_ids=list(range(NCORES)), trace=trace
    )
    if res.exec_time_ns is not None:
        LAST_EXEC_NS = res.exec_time_ns
    outs = []
    for i, r in enumerate(res.results):
        s = slice(i * PER, (i + 1) * PER)
        outs.append(
            postprocess_core_output(
                r["out"], meta_knowledge[s], x_all[s], w2_b, bl_w, bl_b
            )
        )
    out = np.concatenate(outs, axis=0)
    return out.reshape(B, N, LOUT, O)


# revision 8
# speedup vs baseline: 1.0532x; 1.0041x over previous
"""MetaConv1d Trainium2 kernel — v17 (pair-packed matmul1).

Math (per sample n):
  W1c[d, c] = sum_m meta_aug[m, n] * w1aug[m, (d, c)]   (step1, tile-batched)
  G[e, t]   = sum_c W1c[e, c] * x[c, t]                 (matmul1)
  out[t, o] = sum_{e, j} G[e, t+j] * w2[e, (j, o)]      (matmul2, 3 taps)
Host adds the two cheap bias terms (bl linear; w2-bias x channel-sum conv).

v17 changes vs v15:
  - matmul1 is PAIR-PACKED: one K=128 matmul per sample PAIR using a
    block-diagonal lhsT W1cTp[(par*64 + c), (e2*NP + q)] where e2 =
    d + 32*par; even sample's W1cT occupies rows 0:64 / e2 0:32, odd
    rows 64:128 / e2 32:64, zeros elsewhere (zero-DMA'd once at start).
    This halves matmul1 PE time (64 -> 32 cycles/sample... N=128/pair).
  - psG packs 16 samples per PSUM bank [128, 512] (pairs at row-halves
    x 4 column blocks); one evac copy per bank into G_sb.
  - metaT is host-reordered parity-major per tile so the transpose
    evacuation copies have contiguous innermost dims (DVE 2x mode).
  - matmul2 is QUAD-BATCHED: each G_sb column block holds 4 consecutive
    samples' G rows (32 each); one K=128 matmul per tap with a [128, 256]
    block-diagonal w2 rhs computes all 4 samples (N=256 -> 64/sample/tap).
  - out DMA on the SP queue, x DMA on gpsimd: DMA queues overlap
    (concurrent DMAs from different issuing engines run in parallel).
  - hypernet stage is software-pipelined one tile ahead of m1/m2; PE
    warmup matmuls cover the p-state ramp and initial DMA latency.

Sharding: batch*node dim (6624) split evenly over 8 cores (828 each).
"""

import numpy as np
import ml_dtypes

import concourse.mybir as mybir
import concourse.bacc as bacc
from concourse.tile import TileContext
from concourse.bass_utils import run_bass_kernel_spmd

BF16 = mybir.dt.bfloat16
F32 = mybir.dt.float32

B = 32
N = 207
BN = B * N            # 6624
L = 128
C = 64                # in channels
O = 64                # out channels
KK = 3
META = 32
MA = META + 1         # aug (ones row feeds w1 bias in step1)
LOUT = L - KK + 1     # 126
NCORES = 8
PER = BN // NCORES    # 828
NTS = 128             # samples per tile
NP = NTS // 2         # pairs per full tile (W1cTp column stride)


def tiles_for(per, nts=NTS):
    # small first tile (fast pipeline start) and small last tile (short
    # drain); middles of size nts
    if per <= nts:
        return [(0, per)]
    rem = per
    k = 0
    while rem - nts >= 48:
        rem -= nts
        k += 1
    sizes = [rem] + [nts] * k
    out = []
    n0 = 0
    for s in sizes:
        out.append((n0, s))
        n0 += s
    return out


def build_program(per=PER, nts=NTS):
    assert per % 4 == 0
    npairs = nts // 2
    nc = bacc.Bacc("TRN2", target_bir_lowering=False)

    # x image: partition p = (n%2)*64 + c ; col = (n//2)*L + t
    x_d = nc.dram_tensor("x", (128, (per // 2) * L), BF16, kind="ExternalInput")
    # metaT: per-tile parity-major 128-column blocks, zero-padded so step1
    # fills all 128 W1out rows (pad samples compute garbage weights that
    # land in unread W1cTp columns q >= np_)
    n_tiles0 = tiles_for(per, nts)
    metaT_d = nc.dram_tensor(
        "metaT", (MA, 128 * len(n_tiles0)), BF16, kind="ExternalInput"
    )
    # w1augP: [m, d*64 + c] = W1[(c,d), m] (+ w1_b row)
    w1augP_d = nc.dram_tensor("w1augP", (MA, META * C), BF16, kind="ExternalInput")
    # w2 quad-rhs: per tap j a [128, 256] block-diagonal (rows 32s:32s+32
    # -> cols 64s:64s+64 = w2P[:, j-block]); one K=128 matmul computes 4
    # consecutive samples (G_sb stacks their G rows in one column block)
    w2quad_d = nc.dram_tensor("w2quad", (128, KK * 4 * O), BF16, kind="ExternalInput")
    identB_d = nc.dram_tensor("identB", (128, 128), BF16, kind="ExternalInput")
    zeros_d = nc.dram_tensor("zeros", (64, META * npairs), BF16, kind="ExternalInput")
    # out image: [t, n*64 + o] (bf16; host upcasts + adds biases)
    out_d = nc.dram_tensor("out", (LOUT, per * O), BF16, kind="ExternalOutput")

    n_tiles = tiles_for(per, nts)

    with TileContext(nc) as tc:
        with (
            tc.tile_pool(name="const", bufs=1) as cpool,
            tc.tile_pool(name="wpool", bufs=2) as wpool,
            tc.tile_pool(name="xpool", bufs=2) as xpool,
            tc.tile_pool(name="gpool", bufs=2) as gpool,
            tc.tile_pool(name="opool", bufs=2) as opool,
            tc.tile_pool(name="pst", bufs=2, space="PSUM") as pst,
            tc.tile_pool(name="psg", bufs=3, space="PSUM") as psg,
            tc.tile_pool(name="pso", bufs=3, space="PSUM") as pso,
        ):
            # Act issues no DMAs: a DMA occupies the issuing engine for the
            # whole transfer, and Act/DVE are the evacuation workhorses.
            w1augP = cpool.tile([MA, META * C], BF16)
            nc.sync.dma_start(w1augP[:, :], w1augP_d[:, :])
            identB = cpool.tile([128, 128], BF16)
            nc.gpsimd.dma_start(identB[:, :], identB_d[:, :])
            w2quad = cpool.tile([128, KK * 4 * O], BF16)
            w1ctps = []
            for i in range(2):
                w1ctp_i = cpool.tile([128, 64 * npairs], BF16, tag=f"w1ctp{i}")
                w1ctps.append(w1ctp_i)

            def emit_warmup():
                # keep the PE busy from t=0 so the p-state ramp (full clock
                # only after 3us of continuous execution) completes before
                # real matmuls start; also bridges the initial DMA latency.
                wsrc = cpool.tile([128, 512], BF16, tag="warm")
                nc.vector.memset(wsrc[:, :], 0)
                for i in range(4):
                    psW = pst.tile([128, 512], F32, tag="pstile", padded_shape=[128, 512])
                    nc.tensor.matmul(
                        psW[:, :], wsrc[:, 0:128], wsrc[:, :], start=True, stop=True
                    )

            def emit_late_consts():
                # issued after tile 0's input DMAs so they don't delay step1;
                # zeros ride the Pool queue behind the x loads
                nc.sync.dma_start(w2quad[:, :], w2quad_d[:, :])
                for i, t_ in enumerate(w1ctps):
                    eng = nc.gpsimd if i else nc.sync
                    eng.dma_start(t_[64:128, 0 : 32 * npairs], zeros_d[:, :])
                    eng.dma_start(t_[0:64, 32 * npairs : 64 * npairs], zeros_d[:, :])

            def emit_hyper(ti):
                n0, nt = n_tiles[ti]
                np_ = nt // 2
                metaT_sb = wpool.tile([MA, 128], BF16, tag="metaT", padded_shape=[MA, 128])
                nc.sync.dma_start(metaT_sb[:, :], metaT_d[:, 128 * ti : 128 * ti + 128])

                x_sb = xpool.tile(
                    [128, np_ * L], BF16, tag="xsb", padded_shape=[128, npairs * L]
                )
                nc.gpsimd.dma_start(
                    x_sb[:, :], x_d[:, (n0 // 2) * L : ((n0 + nt) // 2) * L]
                )

                W1cTp = w1ctps[ti % 2]

                # step1: W1out[n', (d, c)] batched over the tile.
                # Fixed 128 partitions: the DMA-transpose reads all rows
                # (pad rows are zeroed once at startup).
                W1out = wpool.tile(
                    [128, META * C], BF16, tag="w1out", padded_shape=[128, META * C]
                )
                for k in range(META * C // 512):
                    ps1 = pst.tile([128, 512], F32, tag="pstile", padded_shape=[128, 512])
                    nc.tensor.matmul(
                        ps1[:, :],
                        metaT_sb[:, :],
                        w1augP[:, k * 512 : (k + 1) * 512],
                        start=True,
                        stop=True,
                    )
                    if k % 2:
                        nc.vector.tensor_copy(W1out[:, k * 512 : (k + 1) * 512], ps1[:, :])
                    else:
                        nc.scalar.copy(W1out[:, k * 512 : (k + 1) * 512], ps1[:, :])

                # transpose W1out (n', (d, c)) -> (c + 64*dpar, dpair, n');
                # evac into the block-diagonal e2-major W1cTp layout.
                dstv = W1cTp[:, :].rearrange("p (e q) -> p e q", q=npairs)
                if ti == 0:
                    # PE-transpose path: keeps tile 0's pre-m1 chain short
                    for d0 in range(0, META, 16):
                        psT = pst.tile(
                            [128, 8 * nt], BF16, tag="pstile", padded_shape=[128, 8 * nts]
                        )
                        for k in range(8):
                            nc.tensor.transpose(
                                psT[:, k * nt : (k + 1) * nt],
                                W1out[0:nt, (d0 + 2 * k) * C : (d0 + 2 * k + 2) * C],
                                identB[0:nt, 0:nt],
                            )
                        srcv = psT[:, :].rearrange("p (k n) -> p k n", k=8)
                        for hp in (0, 1):
                            for par in (0, 1):
                                src = srcv[64 * hp : 64 * hp + 64, :, par * np_ : (par + 1) * np_]
                                e0 = d0 + hp + 32 * par
                                dst = dstv[64 * par : 64 * par + 64, e0 : e0 + 15 : 2, 0:np_]
                                nc.vector.tensor_copy(dst, src)
                else:
                    # xbar DMA-transpose: one instruction moves the whole
                    # tile's transpose to the DMA engines (16x128 tiles);
                    # out[p, b, r] = W1out[r, 128b + p]
                    psT_sb = wpool.tile(
                        [128, 16 * 128], BF16, tag="ptsb", padded_shape=[128, 16 * 128]
                    )
                    nc.sync.dma_start_transpose(
                        psT_sb[:, :].rearrange("p (k n) -> p k n", k=16),
                        W1out[:, :],
                    )
                    srcv = psT_sb[:, :].rearrange("p (k n) -> p k n", k=16)
                    for hp in (0, 1):
                        for par in (0, 1):
                            src = srcv[64 * hp : 64 * hp + 64, :, par * np_ : (par + 1) * np_]
                            e0 = hp + 32 * par
                            dst = dstv[64 * par : 64 * par + 64, e0 : e0 + 31 : 2, 0:np_]
                            nc.vector.tensor_copy(dst, src)

                return (n0, nt, x_sb, W1cTp)

            def emit_m1m2(state, cstep=16):
                n0, nt, x_sb, W1cTp = state
                np_ = nt // 2
                # strided lhsT view: [p, q, e2]
                W1r = W1cTp[:, :].rearrange("p (e q) -> p q e", q=npairs)

                G_sb = gpool.tile(
                    [128, ((np_ + 7) // 8) * 512], BF16, tag="gsb",
                    padded_shape=[128, ((npairs + 7) // 8) * 512],
                )
                out_sb = opool.tile(
                    [LOUT, nt * O], BF16, tag="osb", padded_shape=[LOUT, nts * O]
                )
                out_p = out_sb[:, :].rearrange("t (n z) -> t n z", z=2 * O)

                # c-blocks of pairs (even starts); m1 pipelined one block
                # ahead of m2
                cblocks = [(c0, min(c0 + cstep, np_)) for c0 in range(0, np_, cstep)]

                def emit_m1(c0, ce):
                    for g0 in range(c0, ce, 8):
                        ge = min(g0 + 8, ce)
                        gw = ((ge - g0 + 1) // 2) * 128
                        psG = psg.tile(
                            [128, gw], F32, tag="psG",
                            padded_shape=[128, 512],
                        )
                        for q in range(g0, ge):
                            qq = q - g0
                            nc.tensor.matmul(
                                psG[64 * (qq % 2) : 64 * (qq % 2) + 64,
                                    128 * (qq // 2) : 128 * (qq // 2) + 128],
                                W1r[:, q, :],
                                x_sb[:, q * L : (q + 1) * L],
                                start=True,
                                stop=True,
                            )
                        gb = (g0 // 8) * 512 + ((g0 % 8) // 2) * 128
                        dst = G_sb[:, gb : gb + gw]
                        nc.scalar.copy(dst, psG[:, :])

                def emit_m2(c0, ce):
                    # per QUAD (pairs q0, q0+1 share a G_sb column block with
                    # 4 consecutive samples' G at rows 32s): 3 accumulating
                    # K=128 matmuls with block-diagonal w2 rhs [128, 256]
                    quads = list(range(c0, ce, 2))
                    for b0 in range(0, len(quads), 2):
                        qb = quads[b0 : b0 + 2]
                        psO = pso.tile(
                            [LOUT, len(qb) * 4 * O], F32, tag="psO",
                            padded_shape=[LOUT, 8 * O],
                        )
                        for k, q0 in enumerate(qb):
                            gcol = (q0 // 8) * 512 + ((q0 % 8) // 2) * 128
                            for j in range(KK):
                                nc.tensor.matmul(
                                    psO[:, k * 4 * O : (k + 1) * 4 * O],
                                    G_sb[:, gcol + j : gcol + j + LOUT],
                                    w2quad[:, j * 4 * O : (j + 1) * 4 * O],
                                    start=(j == 0),
                                    stop=(j == KK - 1),
                                )
                        # samples 2*qb[0] .. 2*qb[-1]+3 are consecutive
                        nb = 2 * qb[0]
                        ns_ = 4 * len(qb)
                        dst = out_sb[:, nb * O : (nb + ns_) * O]
                        if (b0 // 2) % 2 == 0:
                            nc.vector.tensor_copy(dst, psO[:, :])
                        else:
                            nc.scalar.copy(dst, psO[:, :])


                emit_m1(*cblocks[0])
                for ci, (c0, ce) in enumerate(cblocks):
                    if ci + 1 < len(cblocks):
                        emit_m1(*cblocks[ci + 1])
                    emit_m2(c0, ce)
                    nc.sync.dma_start(
                        out_d[:, (n0 + 2 * c0) * O : (n0 + 2 * ce) * O],
                        out_sb[:, 2 * c0 * O : 2 * ce * O],
                    )

            # software pipeline: hypernet one tile ahead of m1/m2
            emit_warmup()
            prev = emit_hyper(0)
            emit_late_consts()
            for ti in range(1, len(n_tiles)):
                nxt = emit_hyper(ti)
                emit_m1m2(prev)
                prev = nxt
            emit_m1m2(prev, cstep=8)
    if not nc.is_finalized():
        nc.finalize()
    return nc


def _host_prep(w1_w, w1_b, w2_w, npairs):
    bf = ml_dtypes.bfloat16
    # w1augP[m, (d, c)] = W1[(c*META+d), m]; row 32 = w1_b
    w1 = w1_w.reshape(C, META, META).transpose(2, 1, 0)      # (m, d, c)
    w1b = w1_b.reshape(C, META).T                            # (d, c)
    w1aug = np.concatenate([w1, w1b[None]], axis=0)          # (33, d, c)
    w1augP = w1aug.reshape(MA, META * C)
    # w2P[e, (j, o)] = w2_w[(o*KK+j), e], e < 32; replicated at 4 bases
    w2 = w2_w.reshape(O, KK, META).transpose(2, 1, 0)        # (e, j, o)
    w2P = w2.reshape(META, KK * O)
    w2quad = np.zeros((128, KK * 4 * O), np.float32)
    for j in range(KK):
        for s_ in range(4):
            w2quad[32 * s_ : 32 * s_ + 32, j * 256 + 64 * s_ : j * 256 + 64 * s_ + 64] = \
                w2P[:, j * 64 : (j + 1) * 64]
    identB = np.eye(128, dtype=bf)
    zeros = np.zeros((64, META * npairs), dtype=bf)
    return w1augP.astype(bf), w2quad.astype(bf), identB, zeros


def make_core_inputs(meta, x, w1_w, w1_b, w2_w, w2_b, nts=NTS):
    """meta (per, 32) f32, x (per, L, C) f32 -> input map for one core."""
    bf = ml_dtypes.bfloat16
    per = meta.shape[0]
    w1augP, w2quad, identB, zeros = _host_prep(w1_w, w1_b, w2_w, nts // 2)
    # per-tile parity-major column order for metaT
    tls = tiles_for(per, nts)
    metaT = np.zeros((MA, 128 * len(tls)), np.float32)
    for ti, (n0, nt) in enumerate(tls):
        perm = list(range(n0, n0 + nt, 2)) + list(range(n0 + 1, n0 + nt, 2))
        metaT[:META, 128 * ti : 128 * ti + nt] = meta[np.array(perm)].T
        metaT[META, 128 * ti : 128 * ti + nt] = 1.0
    metaT = metaT.astype(bf)
    # x image: [ (n%2)*64 + c, (n//2)*L + t ]
    xt = np.ascontiguousarray(x.transpose(0, 2, 1)).astype(bf)   # (per, C, L)
    ximg = xt.reshape(per // 2, 2, C, L).transpose(1, 2, 0, 3).reshape(128, (per // 2) * L)
    return {
        "x": np.ascontiguousarray(ximg),
        "metaT": np.ascontiguousarray(metaT),
        "w1augP": w1augP,
        "w2quad": w2quad,
        "identB": identB,
        "zeros": zeros,
    }


def postprocess_core_output(out_raw, meta, x, w2_b, bl_w=None, bl_b=None):
    """out_raw (LOUT, per*O) bf16 -> (per, LOUT, O) f32 with host bias terms."""
    per = meta.shape[0]
    out = np.asarray(out_raw, dtype=np.float32).reshape(LOUT, per, O).transpose(1, 0, 2)
    # w2 bias term: out[t, o] += sum_j b2[(o,j)] * s[t+j], s = channel sum
    s = x.sum(axis=2)                                        # (per, L)
    b2 = w2_b.reshape(O, KK)                                 # (o, j)
    sw = np.lib.stride_tricks.sliding_window_view(s, KK, axis=1)  # (per, LOUT, KK)
    out = out + sw @ b2.T                                    # (per, LOUT, O)
    if bl_w is not None:
        b = meta @ bl_w.T + bl_b                             # (per, O)
        out = out + b[:, None, :]
    return np.ascontiguousarray(out)


LAST_EXEC_NS = None
_NC_CACHE = {}


def kernel(meta_knowledge, input, w1_w, w1_b, w2_w, w2_b, bl_w, bl_b):
    global LAST_EXEC_NS
    import os

    x_all = np.ascontiguousarray(input.reshape(BN, L, C), dtype=np.float32)

    if PER not in _NC_CACHE:
        _NC_CACHE[PER] = build_program(PER)
    nc = _NC_CACHE[PER]
    in_maps = []
    for i in range(NCORES):
        s = slice(i * PER, (i + 1) * PER)
        in_maps.append(
            make_core_inputs(meta_knowledge[s], x_all[s], w1_w, w1_b, w2_w, w2_b)
        )
    trace = os.environ.get("KM_TRACE", "0") == "1"
    res = run_bass_kernel_spmd(
        nc, in_maps, core_ids=list(range(NCORES)), trace=trace
    )
    if res.exec_time_ns is not None:
        LAST_EXEC_NS = res.exec_time_ns
    outs = []
    for i, r in enumerate(res.results):
        s = slice(i * PER, (i + 1) * PER)
        outs.append(
            postprocess_core_output(
                r["out"], meta_knowledge[s], x_all[s], w2_b, bl_w, bl_b
            )
        )
    out = np.concatenate(outs, axis=0)
    return out.reshape(B, N, LOUT, O)
